# revision 27
# baseline (speedup 1.0000x reference)
"""Embedding gather (DirectCXLEmbedding) on 8 TRN2 NeuronCores.

Design (vocab-sharded + dedup + greedy pair-coalesced int16 SWDGE gather):

1. Vocab (table) sharding: core i owns table rows [i*125000, (i+1)*125000)
   and handles the indices landing in its shard (~102,400 of the global
   819,200 for uniform inputs).  The host routes indices to owner cores by
   sorting them once; the "all-to-all" of classic vocab-sharded embeddings
   is free because kernel() owns full inputs and outputs anyway.  Each core
   only receives its 32 MB table slice.

2. Dedup: at 0.82 draws/row, ~32% of a core's sorted indices are
   duplicates.  The device gathers each unique row once (~70,000 rows/core);
   the host expands duplicates during the same fancy-index that inverts the
   sort.

3. Greedy pair coalescing: unique rows are dense in the shard (~0.56/row).
   Greedy pairing of adjacent unique rows covers ~72% of them; each pair
   moves as ONE 512-B gather element (elem_size=128 f32), halving its
   descriptor count and clearing the sub-512B DMA penalty on both the HBM
   read and SBUF write side.  Pairs starting at even rows use the table
   viewed as [62500, 128]; pairs starting at odd rows use the same view
   shifted one row; leftovers go through a 256-B single-row stream.
   ~45K gather elements/core instead of 102K naive.

4. Gather engine: GPSIMD `dma_gather` (InstDMAGatherAnt, SWDGE) gathers up
   to 1024 elements per instruction (HW limit found empirically; >1024
   crashes the device) by int16 index.  Each stream is cut into chunks of
   <=1024 sorted elements; chunk c reads from a STATIC 32,768-row window
   based at the expected rank-quantile minus margin, so chunk-local indices
   fit int16 with large slack.  Out-of-window elements (non-uniform inputs)
   spill to a host-side numpy gather — zero spills for the target workload.

5. Device pipeline: per chunk, one full-capacity dma_gather (unused slots
   carry a dummy in-window index 0, so every staging lane is written — no
   staging memset, no valid-count plumbing) into an SBUF staging slot, then
   a contiguous HWDGE store from SP.  Gathers (GPSIMD/SWDGE) and stores
   (SP/HWDGE) overlap; staging slots rotate over NBUF per-slot semaphore
   pairs (a DMA's "+16" is 16 independent +1s from the SDMA engines, so a
   semaphore is only safely waitable with a single DMA in flight on it).
"""

import numpy as np

# Problem constants (hardcoded per harness contract).
B, L = 16384, 50
V, D = 1_000_000, 64
N_CORES = 8
P = 128
N_FLAT = B * L                            # 819,200 total gathers

SHARD = V // N_CORES                      # 125,000 table rows per core
CHUNK = 1024                              # max num_idxs per dma_gather
S = CHUNK // 16                           # int16 idx columns per chunk (64)
J = CHUNK // 128                          # elements per partition per chunk (8)
WIN = 1 << 15                             # int16 window (32768 rows)

# chunk counts per stream (sized to the uniform workload's per-core maxima
# plus ~5 sigma; out-of-capacity inputs spill to the host path)
NCH_E = 13                                # even-aligned pair chunks
NCH_O = 13                                # odd-aligned pair chunks
NCH_S = 20                                # single chunks
NCHT = NCH_E + NCH_O + NCH_S              # 46 gather instructions per core

PAIR_RANGE = SHARD // 2                   # pair-unit address space (62,500)
WIN_P = WIN // 2                          # window in pair units (16,384)
_E_PAIR = 12_600                          # expected pairs per alignment
_E_SNGL = 19_800                          # expected singles per core

BASES_E = np.clip(
    (np.arange(NCH_E) * CHUNK * PAIR_RANGE) // _E_PAIR - 3_000,
    0,
    PAIR_RANGE - WIN_P,
).astype(np.int64)
BASES_O = np.clip(
    (np.arange(NCH_O) * CHUNK * PAIR_RANGE) // _E_PAIR - 3_000,
    0,
    PAIR_RANGE - WIN_P - 1,
).astype(np.int64)
BASES_S = np.clip(
    (np.arange(NCH_S) * CHUNK * SHARD) // _E_SNGL - 6_000,
    0,
    SHARD - WIN,
).astype(np.int64)

NBUF = 16                                 # staging slots (4 KB/partition each)
SLOT = J * 2 * D                          # slot stride in f32 (pair-chunk size)


def _build_module():
    from contextlib import ExitStack

    import concourse.bacc as bacc
    import concourse.mybir as mybir

    nc = bacc.Bacc()

    idxs = nc.dram_tensor("idxs", [P, NCHT * S], mybir.dt.int16, kind="ExternalInput")
    weight = nc.dram_tensor("weight", [SHARD, D], mybir.dt.float32, kind="ExternalInput")
    out_p = nc.dram_tensor(
        "out_p", [NCH_E + NCH_O, P, J * 2 * D], mybir.dt.float32,
        kind="ExternalOutput",
    )
    out_s = nc.dram_tensor(
        "out_s", [NCH_S, P, J * D], mybir.dt.float32, kind="ExternalOutput"
    )

    with ExitStack() as ctx:
        idx_sb = ctx.enter_context(nc.sbuf_tensor([P, NCHT * S], mybir.dt.int16))
        stage = ctx.enter_context(
            nc.sbuf_tensor([P, NBUF * SLOT], mybir.dt.float32)
        )
        ld_sem = ctx.enter_context(nc.semaphore("ld_sem"))
        ig_sems = [
            ctx.enter_context(nc.semaphore(f"ig{t}")) for t in range(NBUF)
        ]
        st_sems = [
            ctx.enter_context(nc.semaphore(f"st{t}")) for t in range(NBUF)
        ]
        block = ctx.enter_context(nc.Block())

        @block.gpsimd
        def _(g):
            g.dma_start(out=idx_sb[:], in_=idxs[:]).then_inc(ld_sem, 16)
            g.wait_ge(ld_sem, 16)
            for c in range(NCHT):
                slot = c % NBUF
                if c >= NBUF:
                    # staging slot must have been stored out (same-lane store)
                    g.wait_ge(st_sems[slot], 16 * (c // NBUF))
                if c < NCH_E + NCH_O:     # pair chunk: 512-B elements
                    if c < NCH_E:
                        row0 = int(BASES_E[c]) * 2
                    else:
                        row0 = int(BASES_O[c - NCH_E]) * 2 + 1
                    in_ap = weight[row0:row0 + WIN, :].rearrange(
                        "(a two) d -> a (two d)", two=2
                    )
                    out_ap = stage[:, slot * SLOT:(slot + 1) * SLOT].rearrange(
                        "p (j d) -> p j d", d=2 * D
                    )
                    elem = 2 * D
                else:                     # single chunk: 256-B elements
                    row0 = int(BASES_S[c - NCH_E - NCH_O])
                    in_ap = weight[row0:row0 + WIN, :]
                    out_ap = stage[
                        :, slot * SLOT:slot * SLOT + J * D
                    ].rearrange("p (j d) -> p j d", d=D)
                    elem = D
                g.dma_gather(
                    out_ap=out_ap,
                    in_ap=in_ap,
                    idxs_ap=idx_sb[:, c * S:(c + 1) * S],
                    num_idxs=CHUNK,
                    num_idxs_reg=CHUNK,
                    elem_size=elem,
                ).then_inc(ig_sems[slot], 16)

        @block.sync
        def _(s):
            for c in range(NCHT):
                slot = c % NBUF
                s.wait_ge(ig_sems[slot], 16 * (c // NBUF + 1))
                if c < NCH_E + NCH_O:
                    s.dma_start(
                        out=out_p[c, :, :],
                        in_=stage[:, slot * SLOT:(slot + 1) * SLOT],
                    ).then_inc(st_sems[slot], 16)
                else:
                    s.dma_start(
                        out=out_s[c - NCH_E - NCH_O, :, :],
                        in_=stage[:, slot * SLOT:slot * SLOT + J * D],
                    ).then_inc(st_sems[slot], 16)
            for c in range(NCHT - NBUF, NCHT):
                slot = c % NBUF
                s.wait_ge(st_sems[slot], 16 * (c // NBUF + 1))

    nc.compile()
    return nc


_NC_CACHE = None


def _chunk_stream(vals: np.ndarray, bases: np.ndarray, nch: int, win: int):
    """Pack sorted element values into nch chunks of CHUNK int16 slots.

    Unused slots get dummy index 0 (in-window), so the device always gathers
    full chunks and every staging lane is written.  Returns (buf [nch,
    CHUNK] int16, valid mask over vals' ranks — True iff gathered)."""
    cap = nch * CHUNK
    n = len(vals)
    take = min(n, cap)
    pad = np.full(cap, -1, dtype=np.int64)
    pad[:take] = vals[:take]
    chunks = pad.reshape(nch, CHUNK)
    rel = chunks - bases[:, None]
    in_win = (rel >= 0) & (rel < win) & (chunks >= 0)

    buf = np.zeros((nch, CHUNK), dtype=np.int16)         # dummy idx 0
    nval = in_win.sum(axis=1)
    for c in range(nch):
        buf[c, :nval[c]] = rel[c][in_win[c]].astype(np.int16)

    valid = np.zeros(n, dtype=bool)
    valid[:take] = in_win.reshape(-1)[:take]
    return buf, valid


def _wrap16(buf: np.ndarray) -> np.ndarray:
    """[nch, CHUNK] -> 16-partition-wrapped, 8x-replicated [P, nch*S]."""
    nch = buf.shape[0]
    idx16 = buf.reshape(nch, S, 16).transpose(0, 2, 1)   # [nch, 16, S]
    idx16 = np.tile(idx16, (1, 8, 1))                    # [nch, 128, S]
    return np.ascontiguousarray(idx16.transpose(1, 0, 2).reshape(P, nch * S))


def _scatter_stream(full_u, filled, rows, ranks, valid, nch, two):
    """Write device rows of one stream into full_u at the streams' u-ranks.

    rows: [nch, CHUNK, elem_D]; ranks: u-rank per stream element; valid:
    gathered mask per stream element (device rows form compacted prefixes
    per chunk)."""
    n = len(ranks)
    for c in range(nch):
        lo_e, hi_e = c * CHUNK, min((c + 1) * CHUNK, n)
        if lo_e >= n:
            break
        vm = valid[lo_e:hi_e]
        k = int(vm.sum())
        if k == 0:
            continue
        ru = ranks[lo_e + vm.nonzero()[0]]
        if two:
            full_u[ru] = rows[c, :k, :D]
            full_u[ru + 1] = rows[c, :k, D:]
            filled[ru] = True
            filled[ru + 1] = True
        else:
            full_u[ru] = rows[c, :k]
            filled[ru] = True


def kernel(indices: np.ndarray, weight: np.ndarray) -> np.ndarray:
    global _NC_CACHE
    from concourse.bass_utils import run_bass_kernel_spmd

    indices = np.asarray(indices)
    weight = np.ascontiguousarray(np.asarray(weight, dtype=np.float32))
    assert indices.shape == (B, L), indices.shape
    assert weight.shape == (V, D), weight.shape

    if _NC_CACHE is None:
        _NC_CACHE = _build_module()
    nc = _NC_CACHE

    gflat = indices.reshape(-1).astype(np.int64)
    g_order = np.argsort(gflat, kind="stable")           # routes + sorts
    sv = gflat[g_order]                                  # ascending values
    bounds = np.searchsorted(sv, np.arange(N_CORES + 1) * SHARD)

    in_maps = []
    metas = []
    for i in range(N_CORES):
        lo, hi = int(bounds[i]), int(bounds[i + 1])
        local = sv[lo:hi] - i * SHARD
        n = len(local)
        if n == 0:
            u = np.empty(0, np.int64)
            u_rank = np.empty(0, np.int64)
        else:
            newv = np.empty(n, dtype=bool)
            newv[0] = True
            np.not_equal(local[1:], local[:-1], out=newv[1:])
            u_rank = np.cumsum(newv) - 1                 # sorted rank -> u rank
            u = local[newv]                              # sorted unique values
        n_u = len(u)

        # greedy pairing of adjacent unique rows (within runs)
        adj_next = np.zeros(n_u, dtype=bool)
        if n_u > 1:
            adj_next[:-1] = u[1:] == u[:-1] + 1
        adj_prev = np.zeros(n_u, dtype=bool)
        adj_prev[1:] = adj_next[:-1]
        run_start = ~adj_prev
        ar = np.arange(n_u)
        first = np.maximum.accumulate(np.where(run_start, ar, -1))
        pairstart = ((ar - first) % 2 == 0) & adj_next
        member = pairstart.copy()
        member[1:] |= pairstart[:-1]

        even_ps = pairstart & (u % 2 == 0)
        odd_ps = pairstart & (u % 2 == 1)
        e_vals = u[even_ps] >> 1                         # pair units
        o_vals = (u[odd_ps] - 1) >> 1
        s_vals = u[~member]
        e_ranks = even_ps.nonzero()[0]
        o_ranks = odd_ps.nonzero()[0]
        s_ranks = (~member).nonzero()[0]

        buf_e, val_e = _chunk_stream(e_vals, BASES_E, NCH_E, WIN_P)
        buf_o, val_o = _chunk_stream(o_vals, BASES_O, NCH_O, WIN_P)
        buf_s, val_s = _chunk_stream(s_vals, BASES_S, NCH_S, WIN)

        idx16 = np.concatenate(
            [_wrap16(buf_e), _wrap16(buf_o), _wrap16(buf_s)], axis=1
        )
        in_maps.append({
            "idxs": idx16,
            "weight": weight[i * SHARD:(i + 1) * SHARD],
        })
        metas.append((lo, hi, u, u_rank,
                      e_ranks, o_ranks, s_ranks, val_e, val_o, val_s))

    res = run_bass_kernel_spmd(nc, in_maps, core_ids=list(range(N_CORES)))

    slot = np.arange(CHUNK)
    result = np.empty((N_FLAT, D), dtype=np.float32)
    for i in range(N_CORES):
        (lo, hi, u, u_rank,
         e_ranks, o_ranks, s_ranks, val_e, val_o, val_s) = metas[i]
        if hi == lo:
            continue
        n_u = len(u)
        full_u = np.empty((n_u, D), dtype=np.float32)
        filled = np.zeros(n_u, dtype=bool)

        arr_p = res.results[i]["out_p"].reshape(NCH_E + NCH_O, P, J, 2 * D)
        rows_p = arr_p[:, slot % 128, slot // 128, :]    # [.., CHUNK, 2D]
        _scatter_stream(full_u, filled, rows_p[:NCH_E], e_ranks, val_e,
                        NCH_E, two=True)
        _scatter_stream(full_u, filled, rows_p[NCH_E:], o_ranks, val_o,
                        NCH_O, two=True)

        arr_s = res.results[i]["out_s"].reshape(NCH_S, P, J, D)
        rows_s = arr_s[:, slot % 128, slot // 128, :]    # [NCH_S, CHUNK, D]
        _scatter_stream(full_u, filled, rows_s, s_ranks, val_s,
                        NCH_S, two=False)

        if not filled.all():                             # spills: host gather
            miss = (~filled).nonzero()[0]
            full_u[miss] = weight[i * SHARD + u[miss]]
        result[g_order[lo:hi]] = full_u[u_rank]

    return result.reshape(B, L, D)


# revision 28
# speedup vs baseline: 1.0328x; 1.0328x over previous
"""Embedding gather (DirectCXLEmbedding) on 8 TRN2 NeuronCores.

Design (vocab-sharded + dedup + greedy pair-coalesced int16 SWDGE gather):

1. Vocab (table) sharding: core i owns table rows [i*125000, (i+1)*125000)
   and handles the indices landing in its shard (~102,400 of the global
   819,200 for uniform inputs).  The host routes indices to owner cores by
   sorting them once; the "all-to-all" of classic vocab-sharded embeddings
   is free because kernel() owns full inputs and outputs anyway.  Each core
   only receives its 32 MB table slice.

2. Dedup: at 0.82 draws/row, ~32% of a core's sorted indices are
   duplicates.  The device gathers each unique row once (~70,000 rows/core);
   the host expands duplicates during the same fancy-index that inverts the
   sort.

3. Greedy pair coalescing: unique rows are dense in the shard (~0.56/row).
   Greedy pairing of adjacent unique rows covers ~72% of them; each pair
   moves as ONE 512-B gather element (elem_size=128 f32), halving its
   descriptor count and clearing the sub-512B DMA penalty on both the HBM
   read and SBUF write side.  Pairs starting at even rows use the table
   viewed as [62500, 128]; pairs starting at odd rows use the same view
   shifted one row; leftovers go through a 256-B single-row stream.
   ~45K gather elements/core instead of 102K naive.

4. Gather engine: GPSIMD `dma_gather` (InstDMAGatherAnt, SWDGE) gathers up
   to 1024 elements per instruction (HW limit found empirically; >1024
   crashes the device) by int16 index.  Each stream is cut into chunks of
   sorted elements (1024 each plus a ragged 512 tail); chunk c reads from a
   STATIC 32,768-row window based at the expected rank-quantile minus
   margin, so chunk-local indices fit int16 with large slack.
   Out-of-window elements (non-uniform inputs) spill to a host-side numpy
   gather — zero spills for the target workload.

5. Device pipeline: per chunk, one full-capacity dma_gather (unused slots
   carry a dummy in-window index 0, so every staging lane is written — no
   staging memset, no valid-count plumbing) into an SBUF staging slot, then
   a contiguous HWDGE store from SP.  Gathers (GPSIMD/SWDGE) and stores
   (SP/HWDGE) overlap; staging slots rotate over NBUF per-slot semaphore
   pairs (a DMA's "+16" is 16 independent +1s from the SDMA engines, so a
   semaphore is only safely waitable with a single DMA in flight on it).
"""

import numpy as np

# Problem constants (hardcoded per harness contract).
B, L = 16384, 50
V, D = 1_000_000, 64
N_CORES = 8
P = 128
N_FLAT = B * L                            # 819,200 total gathers

SHARD = V // N_CORES                      # 125,000 table rows per core
WIN = 1 << 15                             # int16 window (32768 rows)
PAIR_RANGE = SHARD // 2                   # pair-unit address space (62,500)
WIN_P = WIN // 2                          # window in pair units (16,384)

# per-stream chunk schedules (num_idxs per dma_gather; 1024 is the HW max).
# Capacities sized to the uniform workload's per-core maxima (+~1 sigma);
# out-of-capacity/window inputs spill to the host path.
SCHED_E = [1024] * 12 + [512]             # even-aligned pairs (cap 12,800)
SCHED_O = [1024] * 12 + [512]             # odd-aligned pairs  (cap 12,800)
SCHED_S = [1024] * 19 + [512]             # singles            (cap 19,968)

_E_PAIR = 12_600                          # expected pairs per alignment
_E_SNGL = 19_800                          # expected singles per core


def _bases(sched, rng_max, expect, margin, clamp_hi):
    starts = np.concatenate([[0], np.cumsum(sched)[:-1]])
    return np.clip(starts * rng_max // expect - margin, 0, clamp_hi)


BASES_E = _bases(SCHED_E, PAIR_RANGE, _E_PAIR, 3_000, PAIR_RANGE - WIN_P)
BASES_O = _bases(SCHED_O, PAIR_RANGE, _E_PAIR, 3_000, PAIR_RANGE - WIN_P - 1)
BASES_S = _bases(SCHED_S, SHARD, _E_SNGL, 6_000, SHARD - WIN)

NBUF = 16                                 # staging slots (4 KB/partition each)
SLOT = 8 * 2 * D                          # slot stride in f32 (max chunk size)

# flattened chunk table: (stream, idx within stream, num_idxs, kind)
# kind: 0 = even pairs, 1 = odd pairs, 2 = singles
_CHUNKS = (
    [(0, k, n) for k, n in enumerate(SCHED_E)]
    + [(1, k, n) for k, n in enumerate(SCHED_O)]
    + [(2, k, n) for k, n in enumerate(SCHED_S)]
)
NCHT = len(_CHUNKS)
IDX_COLS = sum(n // 16 for _, _, n in _CHUNKS)           # int16 idx columns
PCOLS = sum(n // 128 * 2 * D for _, _, n in _CHUNKS[:len(SCHED_E) + len(SCHED_O)])
SCOLS = sum(n // 128 * D for s, _, n in _CHUNKS if s == 2)


def _build_module():
    from contextlib import ExitStack

    import concourse.bacc as bacc
    import concourse.mybir as mybir

    nc = bacc.Bacc()

    idxs = nc.dram_tensor("idxs", [P, IDX_COLS], mybir.dt.int16, kind="ExternalInput")
    weight = nc.dram_tensor("weight", [SHARD, D], mybir.dt.float32, kind="ExternalInput")
    out_p = nc.dram_tensor("out_p", [P, PCOLS], mybir.dt.float32, kind="ExternalOutput")
    out_s = nc.dram_tensor("out_s", [P, SCOLS], mybir.dt.float32, kind="ExternalOutput")

    with ExitStack() as ctx:
        idx_sb = ctx.enter_context(nc.sbuf_tensor([P, IDX_COLS], mybir.dt.int16))
        stage = ctx.enter_context(
            nc.sbuf_tensor([P, NBUF * SLOT], mybir.dt.float32)
        )
        ld_sem = ctx.enter_context(nc.semaphore("ld_sem"))
        ig_sems = [
            ctx.enter_context(nc.semaphore(f"ig{t}")) for t in range(NBUF)
        ]
        st_sems = [
            ctx.enter_context(nc.semaphore(f"st{t}")) for t in range(NBUF)
        ]
        block = ctx.enter_context(nc.Block())

        # per-chunk precomputed offsets
        icol = np.concatenate([[0], np.cumsum([n // 16 for _, _, n in _CHUNKS])])
        pcol = 0
        scol = 0
        ocols = []
        for s, k, n in _CHUNKS:
            if s in (0, 1):
                ocols.append(pcol)
                pcol += n // 128 * 2 * D
            else:
                ocols.append(scol)
                scol += n // 128 * D

        @block.gpsimd
        def _(g):
            g.dma_start(out=idx_sb[:], in_=idxs[:]).then_inc(ld_sem, 16)
            g.wait_ge(ld_sem, 16)
            for c, (s, k, n) in enumerate(_CHUNKS):
                slot = c % NBUF
                if c >= NBUF:
                    # staging slot must have been stored out (same-lane store)
                    g.wait_ge(st_sems[slot], 16 * (c // NBUF))
                j = n // 128
                if s in (0, 1):           # pair chunk: 512-B elements
                    row0 = (
                        int(BASES_E[k]) * 2 if s == 0
                        else int(BASES_O[k]) * 2 + 1
                    )
                    in_ap = weight[row0:row0 + WIN, :].rearrange(
                        "(a two) d -> a (two d)", two=2
                    )
                    out_ap = stage[
                        :, slot * SLOT:slot * SLOT + j * 2 * D
                    ].rearrange("p (j d) -> p j d", d=2 * D)
                    elem = 2 * D
                else:                     # single chunk: 256-B elements
                    row0 = int(BASES_S[k])
                    in_ap = weight[row0:row0 + WIN, :]
                    out_ap = stage[
                        :, slot * SLOT:slot * SLOT + j * D
                    ].rearrange("p (j d) -> p j d", d=D)
                    elem = D
                g.dma_gather(
                    out_ap=out_ap,
                    in_ap=in_ap,
                    idxs_ap=idx_sb[:, int(icol[c]):int(icol[c + 1])],
                    num_idxs=n,
                    num_idxs_reg=n,
                    elem_size=elem,
                ).then_inc(ig_sems[slot], 16)

        @block.sync
        def _(s_eng):
            for c, (s, k, n) in enumerate(_CHUNKS):
                slot = c % NBUF
                s_eng.wait_ge(ig_sems[slot], 16 * (c // NBUF + 1))
                j = n // 128
                if s in (0, 1):
                    width = j * 2 * D
                    tgt = out_p[:, ocols[c]:ocols[c] + width]
                else:
                    width = j * D
                    tgt = out_s[:, ocols[c]:ocols[c] + width]
                s_eng.dma_start(
                    out=tgt,
                    in_=stage[:, slot * SLOT:slot * SLOT + width],
                ).then_inc(st_sems[slot], 16)
            for c in range(NCHT - NBUF, NCHT):
                slot = c % NBUF
                s_eng.wait_ge(st_sems[slot], 16 * (c // NBUF + 1))

    nc.compile()
    return nc


_NC_CACHE = None


def _chunk_stream(vals: np.ndarray, bases: np.ndarray, sched, win: int):
    """Pack sorted element values into ragged chunks of int16 slots.

    Unused slots get dummy index 0 (in-window), so the device always gathers
    full chunks and every staging lane is written.  Returns (bufs: list of
    [n_c] int16 arrays, valid mask over vals' ranks — True iff gathered)."""
    cap = sum(sched)
    n = len(vals)
    take = min(n, cap)
    pad = np.full(cap, -1, dtype=np.int64)
    pad[:take] = vals[:take]
    valid = np.zeros(n, dtype=bool)

    bufs = []
    off = 0
    for c, n_c in enumerate(sched):
        seg = pad[off:off + n_c]
        rel = seg - bases[c]
        in_win = (rel >= 0) & (rel < win) & (seg >= 0)
        buf = np.zeros(n_c, dtype=np.int16)              # dummy idx 0
        kk = int(in_win.sum())
        buf[:kk] = rel[in_win].astype(np.int16)
        bufs.append(buf)
        lo = off
        hi = min(off + n_c, take)
        if hi > lo:
            valid[lo:hi] = in_win[:hi - lo]
        off += n_c
    return bufs, valid


def _wrap16(buf: np.ndarray) -> np.ndarray:
    """[n_c] slot values -> 16-partition-wrapped, 8x-replicated [P, n_c//16]."""
    sc = len(buf) // 16
    idx16 = buf.reshape(sc, 16).T                        # [16, sc]
    return np.tile(idx16, (8, 1))                        # [128, sc]


def kernel(indices: np.ndarray, weight: np.ndarray) -> np.ndarray:
    global _NC_CACHE
    from concourse.bass_utils import run_bass_kernel_spmd

    indices = np.asarray(indices)
    weight = np.ascontiguousarray(np.asarray(weight, dtype=np.float32))
    assert indices.shape == (B, L), indices.shape
    assert weight.shape == (V, D), weight.shape

    if _NC_CACHE is None:
        _NC_CACHE = _build_module()
    nc = _NC_CACHE

    gflat = indices.reshape(-1).astype(np.int64)
    g_order = np.argsort(gflat, kind="stable")           # routes + sorts
    sv = gflat[g_order]                                  # ascending values
    bounds = np.searchsorted(sv, np.arange(N_CORES + 1) * SHARD)

    in_maps = []
    metas = []
    for i in range(N_CORES):
        lo, hi = int(bounds[i]), int(bounds[i + 1])
        local = sv[lo:hi] - i * SHARD
        n = len(local)
        if n == 0:
            u = np.empty(0, np.int64)
            u_rank = np.empty(0, np.int64)
        else:
            newv = np.empty(n, dtype=bool)
            newv[0] = True
            np.not_equal(local[1:], local[:-1], out=newv[1:])
            u_rank = np.cumsum(newv) - 1                 # sorted rank -> u rank
            u = local[newv]                              # sorted unique values
        n_u = len(u)

        # greedy pairing of adjacent unique rows (within runs)
        adj_next = np.zeros(n_u, dtype=bool)
        if n_u > 1:
            adj_next[:-1] = u[1:] == u[:-1] + 1
        adj_prev = np.zeros(n_u, dtype=bool)
        adj_prev[1:] = adj_next[:-1]
        run_start = ~adj_prev
        ar = np.arange(n_u)
        first = np.maximum.accumulate(np.where(run_start, ar, -1))
        pairstart = ((ar - first) % 2 == 0) & adj_next
        member = pairstart.copy()
        member[1:] |= pairstart[:-1]

        even_ps = pairstart & (u % 2 == 0)
        odd_ps = pairstart & (u % 2 == 1)
        e_vals = u[even_ps] >> 1                         # pair units
        o_vals = (u[odd_ps] - 1) >> 1
        s_vals = u[~member]
        e_ranks = even_ps.nonzero()[0]
        o_ranks = odd_ps.nonzero()[0]
        s_ranks = (~member).nonzero()[0]

        bufs_e, val_e = _chunk_stream(e_vals, BASES_E, SCHED_E, WIN_P)
        bufs_o, val_o = _chunk_stream(o_vals, BASES_O, SCHED_O, WIN_P)
        bufs_s, val_s = _chunk_stream(s_vals, BASES_S, SCHED_S, WIN)

        idx16 = np.concatenate(
            [_wrap16(b) for b in bufs_e + bufs_o + bufs_s], axis=1
        )
        idx16 = np.ascontiguousarray(idx16)
        in_maps.append({
            "idxs": idx16,
            "weight": weight[i * SHARD:(i + 1) * SHARD],
        })
        metas.append((lo, hi, u, u_rank,
                      e_ranks, o_ranks, s_ranks, val_e, val_o, val_s))

    res = run_bass_kernel_spmd(nc, in_maps, core_ids=list(range(N_CORES)))

    def scatter(full_u, filled, flat_dev, sched, ranks, valid, two, col0):
        """flat_dev: [P, cols] device output; chunks at ragged col offsets."""
        n = len(ranks)
        off_e = 0                                        # element offset
        col = col0
        ed = 2 * D if two else D
        for n_c in sched:
            j = n_c // 128
            if off_e < n:
                blk = flat_dev[:, col:col + j * ed].reshape(P, j, ed)
                hi_e = min(off_e + n_c, n)
                vm = valid[off_e:hi_e]
                k = int(vm.sum())
                if k:
                    sl = np.arange(k)
                    rows = blk[sl % 128, sl // 128, :]
                    ru = ranks[off_e + vm.nonzero()[0]]
                    if two:
                        full_u[ru] = rows[:, :D]
                        full_u[ru + 1] = rows[:, D:]
                        filled[ru] = True
                        filled[ru + 1] = True
                    else:
                        full_u[ru] = rows
                        filled[ru] = True
            off_e += n_c
            col += j * ed
        return col

    result = np.empty((N_FLAT, D), dtype=np.float32)
    for i in range(N_CORES):
        (lo, hi, u, u_rank,
         e_ranks, o_ranks, s_ranks, val_e, val_o, val_s) = metas[i]
        if hi == lo:
            continue
        n_u = len(u)
        full_u = np.empty((n_u, D), dtype=np.float32)
        filled = np.zeros(n_u, dtype=bool)

        dev_p = res.results[i]["out_p"]                  # [P, PCOLS]
        dev_s = res.results[i]["out_s"]                  # [P, SCOLS]
        col = scatter(full_u, filled, dev_p, SCHED_E, e_ranks, val_e, True, 0)
        scatter(full_u, filled, dev_p, SCHED_O, o_ranks, val_o, True, col)
        scatter(full_u, filled, dev_s, SCHED_S, s_ranks, val_s, False, 0)

        if not filled.all():                             # spills: host gather
            miss = (~filled).nonzero()[0]
            full_u[miss] = weight[i * SHARD + u[miss]]
        result[g_order[lo:hi]] = full_u[u_rank]

    return result.reshape(B, L, D)


# revision 31
# speedup vs baseline: 1.0663x; 1.0324x over previous
"""Embedding gather (DirectCXLEmbedding) on 8 TRN2 NeuronCores.

Design (vocab-sharded + dedup + greedy pair-coalesced int16 SWDGE gather):

1. Vocab (table) sharding: core i owns table rows [i*125000, (i+1)*125000)
   and handles the indices landing in its shard (~102,400 of the global
   819,200 for uniform inputs).  The host routes indices to owner cores by
   sorting them once; the "all-to-all" of classic vocab-sharded embeddings
   is free because kernel() owns full inputs and outputs anyway.  Each core
   only receives its 32 MB table slice.

2. Dedup: at 0.82 draws/row, ~32% of a core's sorted indices are
   duplicates.  The device gathers each unique row once (~70,000 rows/core);
   the host expands duplicates during the same fancy-index that inverts the
   sort.

3. Greedy pair coalescing: unique rows are dense in the shard (~0.56/row).
   Greedy pairing of adjacent unique rows covers ~72% of them; each pair
   moves as ONE 512-B gather element (elem_size=128 f32), halving its
   descriptor count and clearing the sub-512B DMA penalty on both the HBM
   read and SBUF write side.  Pairs starting at even rows use the table
   viewed as [62500, 128]; pairs starting at odd rows use the same view
   shifted one row; leftovers go through a 256-B single-row stream.
   ~45K gather elements/core instead of 102K naive.

4. Gather engine: GPSIMD `dma_gather` (InstDMAGatherAnt, SWDGE) gathers up
   to 1024 elements per instruction (HW limit found empirically; >1024
   crashes the device) by int16 index.  Each stream is cut into chunks of
   sorted elements (1024 each plus a ragged 512 tail); chunk c reads from a
   STATIC 32,768-row window based at the expected rank-quantile minus
   margin, so chunk-local indices fit int16 with large slack.
   Out-of-window elements (non-uniform inputs) spill to a host-side numpy
   gather — zero spills for the target workload.

5. Device pipeline: per chunk, one full-capacity dma_gather (unused slots
   carry a dummy in-window index 0, so every staging lane is written — no
   staging memset, no valid-count plumbing) into an SBUF staging slot, then
   a contiguous HWDGE store from SP.  Gathers (GPSIMD/SWDGE) and stores
   (SP/HWDGE) overlap; staging slots rotate over NBUF per-slot semaphore
   pairs (a DMA's "+16" is 16 independent +1s from the SDMA engines, so a
   semaphore is only safely waitable with a single DMA in flight on it).
"""

import numpy as np

# Problem constants (hardcoded per harness contract).
B, L = 16384, 50
V, D = 1_000_000, 64
N_CORES = 8
P = 128
N_FLAT = B * L                            # 819,200 total gathers

SHARD = V // N_CORES                      # 125,000 table rows per core
WIN = 1 << 15                             # int16 window (32768 rows)
PAIR_RANGE = SHARD // 2                   # pair-unit address space (62,500)
WIN_P = WIN // 2                          # window in pair units (16,384)

# per-stream chunk schedules (num_idxs per dma_gather; 1024 is the HW max).
# Capacities sized to the uniform workload's per-core maxima (+~1 sigma);
# out-of-capacity/window inputs spill to the host path.
SCHED_T = [1024] * 6 + [256]              # run-end triples    (cap 6,400)
SCHED_E = [1024] * 9 + [384]              # even-aligned pairs (cap 9,600)
SCHED_O = [1024] * 9 + [512]              # odd-aligned pairs  (cap 9,728)
SCHED_S = [1024] * 13 + [512]             # singles            (cap 13,824)

_E_TRIP = 6_150                           # expected triples per core
_E_PAIR = 9_450                           # expected pairs per alignment
_E_SNGL = 13_500                          # expected singles per core


def _bases(sched, rng_max, expect, margin, clamp_hi):
    starts = np.concatenate([[0], np.cumsum(sched)[:-1]])
    return np.clip(starts * rng_max // expect - margin, 0, clamp_hi)


BASES_T = _bases(SCHED_T, SHARD, _E_TRIP, 6_000, SHARD - WIN)
BASES_E = _bases(SCHED_E, PAIR_RANGE, _E_PAIR, 3_000, PAIR_RANGE - WIN_P)
BASES_O = _bases(SCHED_O, PAIR_RANGE, _E_PAIR, 3_000, PAIR_RANGE - WIN_P - 1)
BASES_S = _bases(SCHED_S, SHARD, _E_SNGL, 6_000, SHARD - WIN)

NBUF = 16                                 # staging slots (6 KB/partition each)
SLOT = 8 * 3 * D                          # slot stride in f32 (triple chunks)

# flattened chunk table: (stream, idx within stream, num_idxs)
# stream: 0 = triples, 1 = even pairs, 2 = odd pairs, 3 = singles
_CHUNKS = (
    [(0, k, n) for k, n in enumerate(SCHED_T)]
    + [(1, k, n) for k, n in enumerate(SCHED_E)]
    + [(2, k, n) for k, n in enumerate(SCHED_O)]
    + [(3, k, n) for k, n in enumerate(SCHED_S)]
)
NCHT = len(_CHUNKS)
IDX_COLS = sum(n // 16 for _, _, n in _CHUNKS)           # int16 idx columns
TCOLS = sum(n // 128 * 3 * D for s, _, n in _CHUNKS if s == 0)
PCOLS = sum(n // 128 * 2 * D for s, _, n in _CHUNKS if s in (1, 2))
SCOLS = sum(n // 128 * D for s, _, n in _CHUNKS if s == 3)


def _build_module():
    from contextlib import ExitStack

    import concourse.bacc as bacc
    import concourse.mybir as mybir

    nc = bacc.Bacc()

    idxs = nc.dram_tensor("idxs", [P, IDX_COLS], mybir.dt.int16, kind="ExternalInput")
    weight = nc.dram_tensor("weight", [SHARD, D], mybir.dt.float32, kind="ExternalInput")
    out_t = nc.dram_tensor("out_t", [P, TCOLS], mybir.dt.float32, kind="ExternalOutput")
    out_p = nc.dram_tensor("out_p", [P, PCOLS], mybir.dt.float32, kind="ExternalOutput")
    out_s = nc.dram_tensor("out_s", [P, SCOLS], mybir.dt.float32, kind="ExternalOutput")

    with ExitStack() as ctx:
        idx_sb = ctx.enter_context(nc.sbuf_tensor([P, IDX_COLS], mybir.dt.int16))
        stage = ctx.enter_context(
            nc.sbuf_tensor([P, NBUF * SLOT], mybir.dt.float32)
        )
        ld_sem = ctx.enter_context(nc.semaphore("ld_sem"))
        ig_sems = [
            ctx.enter_context(nc.semaphore(f"ig{t}")) for t in range(NBUF)
        ]
        st_sems = [
            ctx.enter_context(nc.semaphore(f"st{t}")) for t in range(NBUF)
        ]
        block = ctx.enter_context(nc.Block())

        # per-chunk precomputed offsets
        icol = np.concatenate([[0], np.cumsum([n // 16 for _, _, n in _CHUNKS])])
        tcol = pcol = scol = 0
        ocols = []
        for s, k, n in _CHUNKS:
            if s == 0:
                ocols.append(tcol)
                tcol += n // 128 * 3 * D
            elif s in (1, 2):
                ocols.append(pcol)
                pcol += n // 128 * 2 * D
            else:
                ocols.append(scol)
                scol += n // 128 * D

        @block.gpsimd
        def _(g):
            g.dma_start(out=idx_sb[:], in_=idxs[:]).then_inc(ld_sem, 16)
            g.wait_ge(ld_sem, 16)
            for c, (s, k, n) in enumerate(_CHUNKS):
                slot = c % NBUF
                if c >= NBUF:
                    # staging slot must have been stored out (same-lane store)
                    g.wait_ge(st_sems[slot], 16 * (c // NBUF))
                j = n // 128
                if s == 0:                # triple chunk: 768-B elements,
                    row0 = int(BASES_T[k])       # 256-B stride (overlapping AP)
                    win_ap = weight[row0:row0 + WIN, :]
                    import concourse.bass as bass
                    in_ap = bass.AP(
                        win_ap.tensor, win_ap.offset, [[D, WIN - 2], [1, 3 * D]]
                    )
                    out_ap = stage[
                        :, slot * SLOT:slot * SLOT + j * 3 * D
                    ].rearrange("p (j d) -> p j d", d=3 * D)
                    elem = 3 * D
                elif s in (1, 2):         # pair chunk: 512-B elements
                    row0 = (
                        int(BASES_E[k]) * 2 if s == 1
                        else int(BASES_O[k]) * 2 + 1
                    )
                    in_ap = weight[row0:row0 + WIN, :].rearrange(
                        "(a two) d -> a (two d)", two=2
                    )
                    out_ap = stage[
                        :, slot * SLOT:slot * SLOT + j * 2 * D
                    ].rearrange("p (j d) -> p j d", d=2 * D)
                    elem = 2 * D
                else:                     # single chunk: 256-B elements
                    row0 = int(BASES_S[k])
                    in_ap = weight[row0:row0 + WIN, :]
                    out_ap = stage[
                        :, slot * SLOT:slot * SLOT + j * D
                    ].rearrange("p (j d) -> p j d", d=D)
                    elem = D
                g.dma_gather(
                    out_ap=out_ap,
                    in_ap=in_ap,
                    idxs_ap=idx_sb[:, int(icol[c]):int(icol[c + 1])],
                    num_idxs=n,
                    num_idxs_reg=n,
                    elem_size=elem,
                    elem_step=D if s == 0 else None,
                ).then_inc(ig_sems[slot], 16)

        @block.sync
        def _(s_eng):
            for c, (s, k, n) in enumerate(_CHUNKS):
                slot = c % NBUF
                s_eng.wait_ge(ig_sems[slot], 16 * (c // NBUF + 1))
                j = n // 128
                if s == 0:
                    width = j * 3 * D
                    tgt = out_t[:, ocols[c]:ocols[c] + width]
                elif s in (1, 2):
                    width = j * 2 * D
                    tgt = out_p[:, ocols[c]:ocols[c] + width]
                else:
                    width = j * D
                    tgt = out_s[:, ocols[c]:ocols[c] + width]
                s_eng.dma_start(
                    out=tgt,
                    in_=stage[:, slot * SLOT:slot * SLOT + width],
                ).then_inc(st_sems[slot], 16)
            for c in range(NCHT - NBUF, NCHT):
                slot = c % NBUF
                s_eng.wait_ge(st_sems[slot], 16 * (c // NBUF + 1))

    nc.compile()
    return nc


_NC_CACHE = None


def _chunk_stream(vals: np.ndarray, bases: np.ndarray, sched, win: int):
    """Pack sorted element values into ragged chunks of int16 slots.

    Unused slots get dummy index 0 (in-window), so the device always gathers
    full chunks and every staging lane is written.  Returns (bufs: list of
    [n_c] int16 arrays, valid mask over vals' ranks — True iff gathered)."""
    cap = sum(sched)
    n = len(vals)
    take = min(n, cap)
    pad = np.full(cap, -1, dtype=np.int64)
    pad[:take] = vals[:take]
    valid = np.zeros(n, dtype=bool)

    bufs = []
    off = 0
    for c, n_c in enumerate(sched):
        seg = pad[off:off + n_c]
        rel = seg - bases[c]
        in_win = (rel >= 0) & (rel < win) & (seg >= 0)
        buf = np.zeros(n_c, dtype=np.int16)              # dummy idx 0
        kk = int(in_win.sum())
        buf[:kk] = rel[in_win].astype(np.int16)
        bufs.append(buf)
        lo = off
        hi = min(off + n_c, take)
        if hi > lo:
            valid[lo:hi] = in_win[:hi - lo]
        off += n_c
    return bufs, valid


def _wrap16(buf: np.ndarray) -> np.ndarray:
    """[n_c] slot values -> 16-partition-wrapped, 8x-replicated [P, n_c//16]."""
    sc = len(buf) // 16
    idx16 = buf.reshape(sc, 16).T                        # [16, sc]
    return np.tile(idx16, (8, 1))                        # [128, sc]


def kernel(indices: np.ndarray, weight: np.ndarray) -> np.ndarray:
    global _NC_CACHE
    from concourse.bass_utils import run_bass_kernel_spmd

    indices = np.asarray(indices)
    weight = np.ascontiguousarray(np.asarray(weight, dtype=np.float32))
    assert indices.shape == (B, L), indices.shape
    assert weight.shape == (V, D), weight.shape

    if _NC_CACHE is None:
        _NC_CACHE = _build_module()
    nc = _NC_CACHE

    gflat = indices.reshape(-1).astype(np.int64)
    g_order = np.argsort(gflat, kind="stable")           # routes + sorts
    sv = gflat[g_order]                                  # ascending values
    bounds = np.searchsorted(sv, np.arange(N_CORES + 1) * SHARD)

    in_maps = []
    metas = []
    for i in range(N_CORES):
        lo, hi = int(bounds[i]), int(bounds[i + 1])
        local = sv[lo:hi] - i * SHARD
        n = len(local)
        if n == 0:
            u = np.empty(0, np.int64)
            u_rank = np.empty(0, np.int64)
        else:
            newv = np.empty(n, dtype=bool)
            newv[0] = True
            np.not_equal(local[1:], local[:-1], out=newv[1:])
            u_rank = np.cumsum(newv) - 1                 # sorted rank -> u rank
            u = local[newv]                              # sorted unique values
        n_u = len(u)

        # greedy run segmentation: odd runs >=3 end with a 3-row element,
        # the rest is covered by pairs; isolated rows are singles
        adj_next = np.zeros(n_u, dtype=bool)
        if n_u > 1:
            adj_next[:-1] = u[1:] == u[:-1] + 1
        run_start = np.ones(n_u, dtype=bool)
        run_start[1:] = ~adj_next[:-1]
        ar = np.arange(n_u)
        run_id = np.cumsum(run_start) - 1
        rlen = np.bincount(run_id) if n_u else np.zeros(0, np.int64)
        Lr = rlen[run_id] if n_u else np.zeros(0, np.int64)
        first = np.maximum.accumulate(np.where(run_start, ar, -1))
        pos = ar - first
        odd3 = (Lr % 2 == 1) & (Lr >= 3)
        tri_start = odd3 & (pos == Lr - 3)
        pair_end = np.where(odd3, Lr - 3, Lr)
        pairstart = (pos % 2 == 0) & (pos <= pair_end - 2)
        single = Lr == 1

        even_ps = pairstart & (u % 2 == 0)
        odd_ps = pairstart & (u % 2 == 1)
        t_vals = u[tri_start]                            # row units
        e_vals = u[even_ps] >> 1                         # pair units
        o_vals = (u[odd_ps] - 1) >> 1
        s_vals = u[single]
        t_ranks = tri_start.nonzero()[0]
        e_ranks = even_ps.nonzero()[0]
        o_ranks = odd_ps.nonzero()[0]
        s_ranks = single.nonzero()[0]

        bufs_t, val_t = _chunk_stream(t_vals, BASES_T, SCHED_T, WIN - 2)
        bufs_e, val_e = _chunk_stream(e_vals, BASES_E, SCHED_E, WIN_P)
        bufs_o, val_o = _chunk_stream(o_vals, BASES_O, SCHED_O, WIN_P)
        bufs_s, val_s = _chunk_stream(s_vals, BASES_S, SCHED_S, WIN)

        idx16 = np.concatenate(
            [_wrap16(b) for b in bufs_t + bufs_e + bufs_o + bufs_s], axis=1
        )
        idx16 = np.ascontiguousarray(idx16)
        in_maps.append({
            "idxs": idx16,
            "weight": weight[i * SHARD:(i + 1) * SHARD],
        })
        metas.append((lo, hi, u, u_rank, t_ranks,
                      e_ranks, o_ranks, s_ranks, val_t, val_e, val_o, val_s))

    res = run_bass_kernel_spmd(nc, in_maps, core_ids=list(range(N_CORES)))

    def scatter(full_u, filled, flat_dev, sched, ranks, valid, nrows, col0):
        """flat_dev: [P, cols] device output; chunks at ragged col offsets;
        each element carries `nrows` consecutive table rows."""
        n = len(ranks)
        off_e = 0                                        # element offset
        col = col0
        ed = nrows * D
        for n_c in sched:
            j = n_c // 128
            if off_e < n:
                blk = flat_dev[:, col:col + j * ed].reshape(P, j, ed)
                hi_e = min(off_e + n_c, n)
                vm = valid[off_e:hi_e]
                k = int(vm.sum())
                if k:
                    sl = np.arange(k)
                    rows = blk[sl % 128, sl // 128, :]
                    ru = ranks[off_e + vm.nonzero()[0]]
                    for r in range(nrows):
                        full_u[ru + r] = rows[:, r * D:(r + 1) * D]
                        filled[ru + r] = True
            off_e += n_c
            col += j * ed
        return col

    result = np.empty((N_FLAT, D), dtype=np.float32)
    for i in range(N_CORES):
        (lo, hi, u, u_rank, t_ranks,
         e_ranks, o_ranks, s_ranks, val_t, val_e, val_o, val_s) = metas[i]
        if hi == lo:
            continue
        n_u = len(u)
        full_u = np.empty((n_u, D), dtype=np.float32)
        filled = np.zeros(n_u, dtype=bool)

        dev_t = res.results[i]["out_t"]                  # [P, TCOLS]
        dev_p = res.results[i]["out_p"]                  # [P, PCOLS]
        dev_s = res.results[i]["out_s"]                  # [P, SCOLS]
        scatter(full_u, filled, dev_t, SCHED_T, t_ranks, val_t, 3, 0)
        col = scatter(full_u, filled, dev_p, SCHED_E, e_ranks, val_e, 2, 0)
        scatter(full_u, filled, dev_p, SCHED_O, o_ranks, val_o, 2, col)
        scatter(full_u, filled, dev_s, SCHED_S, s_ranks, val_s, 1, 0)

        if not filled.all():                             # spills: host gather
            miss = (~filled).nonzero()[0]
            full_u[miss] = weight[i * SHARD + u[miss]]
        result[g_order[lo:hi]] = full_u[u_rank]

    return result.reshape(B, L, D)
